# revision 6
# baseline (speedup 1.0000x reference)
"""Masked cross-modal attention on 8 Trainium2 NeuronCores.

Reference math (per batch b):
    q,k,v = x @ W{q,k,v}.T   (head-major channels, H=8, Dh=64)
    s     = (q @ k.T) / 8, masked_fill(mask==0, 1e-9), softmax over keys
    out   = (att @ v) @ Wout.T

Masked positions contribute weight exp(1e-9)=1 and value v_j independent of
the query, so with U = unmasked keys, M = masked keys:
    out[t] = (sum_{j in U} e^{s_tj} v_j + sum_{j in M} v_j)
           / (sum_{j in U} e^{s_tj} + |M|)
The kernel runs attention only over gathered unmasked keys (~half) and the
masked-sum corrections are tiny host-side vectors added on-chip.

Sharding: core c -> batch c//2, head-group c%2 (4 of 8 heads). Each core
emits two partial [2048,512] outputs (one per head-pair through its Wout
slice); the host sums the four partials per batch.
"""

import sys

for _p in ("/opt/trn_rl_repo", "/root/.axon_site/_ro/trn_rl_repo"):
    if _p not in sys.path:
        sys.path.append(_p)

import numpy as np
import ml_dtypes
import concourse.bass as bass
import concourse.mybir as mybir
import concourse.tile as tile
from concourse import bacc
from concourse.bass_utils import run_bass_kernel_spmd

F32 = mybir.dt.float32
F32R = mybir.dt.float32r
BF16 = mybir.dt.bfloat16
EXP = mybir.ActivationFunctionType.Exp
ADD = mybir.AluOpType.add
MULT = mybir.AluOpType.mult

# compute dtype for matmul operands (bf16 -> FWL weight loads, half DMA)
CDT = F32R
CDT_NP = ml_dtypes.bfloat16 if CDT == BF16 else np.float32

B, N, DIM = 4, 2048, 512
DL = 256                          # 4 heads * 64 dims per core
SCALE = 64 ** -0.5
TT = N // 512                     # 4 t-tiles of 512
TC = N // 128                     # 16 t-chunks of 128


def _build(nc, s_pad):
    n_sc = s_pad // 128

    xt = nc.dram_tensor("XT", [DIM, N], CDT, kind="ExternalInput")
    xgt = nc.dram_tensor("XGT", [DIM, s_pad], CDT, kind="ExternalInput")
    ind4 = nc.dram_tensor("IND4", [s_pad, 4], CDT, kind="ExternalInput")
    wqt = nc.dram_tensor("WQT", [DIM, DL], CDT, kind="ExternalInput")
    wkt = nc.dram_tensor("WKT", [DIM, DL], CDT, kind="ExternalInput")
    wvt = nc.dram_tensor("WVT", [DIM, DL], CDT, kind="ExternalInput")
    wot = nc.dram_tensor("WOT", [DL, DIM], CDT, kind="ExternalInput")
    corr = nc.dram_tensor("CORR", [65, 4], F32, kind="ExternalInput")
    outs = [nc.dram_tensor(f"OUT{hp}", [N, DIM], F32, kind="ExternalOutput")
            for hp in range(2)]

    with tile.TileContext(nc) as tc:
        with (
            tc.tile_pool(name="persist", bufs=1) as pp,
            tc.tile_pool(name="xpool", bufs=4) as xp,
            tc.tile_pool(name="ps512", bufs=4, space="PSUM") as ps512,
            tc.tile_pool(name="psreg", bufs=2, space="PSUM") as psreg,
            tc.tile_pool(name="epool", bufs=3) as ep,
            tc.tile_pool(name="npool", bufs=3) as np_pool,
            tc.tile_pool(name="dpool", bufs=1) as dpool,
            tc.tile_pool(name="drampool", bufs=4, space="DRAM") as drp,
            tc.tile_pool(name="ahpool", bufs=2) as ahp,
            tc.tile_pool(name="opool", bufs=4) as op,
        ):
            wq_sb = pp.tile([128, 4 * DL], CDT)
            wk_sb = pp.tile([128, 4 * DL], CDT)
            wv_sb = pp.tile([128, 4 * DL], CDT)
            wo_sb = pp.tile([128, 2 * DIM], CDT)
            corr_sb = pp.tile([65, 4], F32)
            qt_sb = pp.tile([128, 2 * N], CDT)           # [d-chunk 2][t]
            kt_sb = pp.tile([128, 2 * s_pad], CDT)       # [d-chunk 2][s]
            v_sb = pp.tile([128, n_sc * 4 * 65], CDT)    # [sc][h][65]
            att_pair = [pp.tile([128, N], CDT, name=f"attp{i}") for i in range(2)]

            # --- input DMAs, critical-path first ---
            for k in range(4):
                nc.sync.dma_start(wk_sb[:, k * DL:(k + 1) * DL], wkt.ap()[k * 128:(k + 1) * 128, :])
                nc.sync.dma_start(wv_sb[:, k * DL:(k + 1) * DL], wvt.ap()[k * 128:(k + 1) * 128, :])
                nc.sync.dma_start(wq_sb[:, k * DL:(k + 1) * DL], wqt.ap()[k * 128:(k + 1) * 128, :])
            nc.sync.dma_start(corr_sb[:], corr.ap())
            v_view = v_sb[:].rearrange("p (s h x) -> p s h x", s=n_sc, h=4)
            for sc in range(n_sc):
                nc.sync.dma_start(v_view[:, sc, :, 64], ind4.ap()[sc * 128:(sc + 1) * 128, :])
            xg_tiles = []
            for k in range(4):
                xg = xp.tile([128, s_pad], CDT, tag="xg")
                nc.sync.dma_start(xg[:], xgt.ap()[k * 128:(k + 1) * 128, :])
                xg_tiles.append(xg)
            xt_tiles = [xp.tile([128, N], CDT, tag="xf", name=f"xf{k}") for k in range(4)]
            for t in range(TT):
                for k in range(4):
                    nc.sync.dma_start(xt_tiles[k][:, t * 512:(t + 1) * 512],
                                      xt.ap()[k * 128:(k + 1) * 128, t * 512:(t + 1) * 512])
            for k in range(2):
                nc.sync.dma_start(wo_sb[:, k * DIM:(k + 1) * DIM], wot.ap()[k * 128:(k + 1) * 128, :])

            s_tiles = [(i * 512, min(512, s_pad - i * 512)) for i in range((s_pad + 511) // 512)]

            def emit_kt(dc):
                for s0, sw in s_tiles:
                    pk = ps512.tile([128, 512], F32, tag="ps512", name="pk")
                    for k in range(4):
                        nc.tensor.matmul(
                            pk[:, :sw],
                            wk_sb[:, k * DL + dc * 128: k * DL + (dc + 1) * 128],
                            xg_tiles[k][:, s0:s0 + sw],
                            start=(k == 0), stop=(k == 3),
                        )
                    nc.vector.tensor_copy(kt_sb[:, dc * s_pad + s0: dc * s_pad + s0 + sw], pk[:, :sw])

            def emit_qt(dc, t):
                pq = ps512.tile([128, 512], F32, tag="ps512", name="pq")
                for k in range(4):
                    nc.tensor.matmul(
                        pq[:],
                        wq_sb[:, k * DL + dc * 128: k * DL + (dc + 1) * 128],
                        xt_tiles[k][:, t * 512:(t + 1) * 512],
                        start=(k == 0), stop=(k == 3),
                    )
                nc.vector.tensor_copy(qt_sb[:, dc * N + t * 512: dc * N + (t + 1) * 512], pq[:])

            def emit_v(sc):
                pv = ps512.tile([128, 256], F32, tag="ps512", name="pv")
                for k in range(4):
                    nc.tensor.matmul(
                        pv[:],
                        xg_tiles[k][:, sc * 128:(sc + 1) * 128],
                        wv_sb[:, k * DL:(k + 1) * DL],
                        start=(k == 0), stop=(k == 3),
                    )
                nc.vector.tensor_copy(
                    v_view[:, sc, :, 0:64],
                    pv[:].rearrange("p (h x) -> p h x", h=4),
                )

            def emit_wout(hp):
                for t in range(TC):
                    po = ps512.tile([128, 512], F32, tag="ps512", name="po")
                    nc.tensor.matmul(
                        po[:],
                        att_pair[hp][:, t * 128:(t + 1) * 128],
                        wo_sb[:, hp * DIM:(hp + 1) * DIM],
                        start=True, stop=True,
                    )
                    o_sb = op.tile([128, 512], F32, tag="o")
                    nc.vector.tensor_copy(o_sb[:], po[:])
                    nc.sync.dma_start(outs[hp].ap()[t * 128:(t + 1) * 128, :], o_sb[:])

            def emit_normalize(hp):
                for h in (2 * hp, 2 * hp + 1):
                    den = dpool.tile([65, N], F32, tag="den")
                    nc.vector.tensor_scalar_add(
                        den[64:65, :],
                        numer_sb[h][64:65, :],
                        corr_sb[64:65, h:h + 1],
                    )
                    scratch = drp.tile([N], F32, tag="scr")
                    nc.sync.dma_start(scratch[:].unsqueeze(0), den[64:65, :])
                    bden = dpool.tile([64, N], F32, tag="bden")
                    nc.sync.dma_start(bden[:], scratch[:].unsqueeze(0).broadcast_to([64, N]))
                    rbc = dpool.tile([64, N], F32, tag="rbc")
                    nc.vector.reciprocal_approx_fast(out=rbc[:], in_=bden[:])
                    att_h = ahp.tile([64, N], CDT, tag="att")
                    nc.vector.scalar_tensor_tensor(
                        out=att_h[:], in0=numer_sb[h][0:64, :],
                        scalar=corr_sb[0:64, h:h + 1], in1=rbc[:],
                        op0=ADD, op1=MULT,
                    )
                    par = (h % 2) * 64
                    nc.sync.dma_start(att_pair[hp][par:par + 64, :], att_h[:])

            emit_kt(0)
            for hp in range(2):
                numer_sb = {}
                for h in (2 * hp, 2 * hp + 1):
                    numer_sb[h] = np_pool.tile([65, N], F32, tag="numer", name=f"numer{h}")
                for t in range(TT):
                    if hp == 0:
                        emit_qt(0, t)
                    pn = {}
                    for h in (2 * hp, 2 * hp + 1):
                        pn[h] = ps512.tile([65, 512], F32, tag="ps512", name=f"pn{h}")
                    # duo groups: (h0,sc),(h1,sc) -> one exp op
                    for sc in range(n_sc):
                        reg = psreg.tile([128, 1024], F32, tag="reg")
                        e_sb = ep.tile([128, 1024], CDT, tag="e")
                        for j, h in enumerate((2 * hp, 2 * hp + 1)):
                            par = (h % 2) * 64
                            nc.tensor.matmul(
                                reg[:, j * 512:(j + 1) * 512],
                                kt_sb[par:par + 64, hp * s_pad + sc * 128: hp * s_pad + (sc + 1) * 128],
                                qt_sb[par:par + 64, hp * N + t * 512: hp * N + (t + 1) * 512],
                                start=True, stop=True,
                            )
                        if hp == 0 and t == 0:
                            emit_v(sc)
                        nc.scalar.activation(e_sb[:], reg[:], EXP, scale=SCALE)
                        for j, h in enumerate((2 * hp, 2 * hp + 1)):
                            nc.tensor.matmul(
                                pn[h][:],
                                v_sb[:, (sc * 4 + h) * 65:(sc * 4 + h + 1) * 65],
                                e_sb[:, j * 512:(j + 1) * 512],
                                start=(sc == 0), stop=(sc == n_sc - 1),
                            )
                    for h in (2 * hp, 2 * hp + 1):
                        nc.vector.tensor_copy(numer_sb[h][:, t * 512:(t + 1) * 512], pn[h][:])
                if hp == 0:
                    emit_kt(1)
                    for t in range(TT):
                        emit_qt(1, t)
                emit_normalize(hp)
                emit_wout(hp)

    nc.compile()
    return nc


def _prep(input_feature, mask, Wq, Wk, Wv, Wout):
    x = np.ascontiguousarray(np.asarray(input_feature, dtype=np.float32))
    m = np.asarray(mask)
    Wq = np.asarray(Wq, dtype=np.float32)
    Wk = np.asarray(Wk, dtype=np.float32)
    Wv = np.asarray(Wv, dtype=np.float32)
    Wout = np.asarray(Wout, dtype=np.float32)

    idxs = [np.flatnonzero(m[b]) for b in range(B)]
    s_pad = max(128, ((max(len(i) for i in idxs) + 127) // 128) * 128)

    def cvt(a):
        return np.ascontiguousarray(a.astype(CDT_NP))

    in_maps = []
    for c in range(8):
        b, g = c // 2, c % 2
        idx = idxs[b]
        cnt = len(idx)
        xg = np.zeros((s_pad, DIM), np.float32)
        xg[:cnt] = x[b][idx]
        ind4 = np.zeros((s_pad, 4), np.float32)
        ind4[:cnt] = 1.0
        xm = x[b][m[b] == 0].sum(axis=0, dtype=np.float32)
        corr = np.zeros((65, 4), np.float32)
        for h in range(4):
            hg = g * 4 + h
            corr[0:64, h] = Wv[hg * 64:(hg + 1) * 64, :] @ xm
            corr[64, h] = np.float32(N - cnt)
        in_maps.append({
            "XT": cvt(x[b].T),
            "XGT": cvt(xg.T),
            "IND4": cvt(ind4),
            "WQT": cvt(Wq[g * DL:(g + 1) * DL, :].T),
            "WKT": cvt(Wk[g * DL:(g + 1) * DL, :].T),
            "WVT": cvt(Wv[g * DL:(g + 1) * DL, :].T),
            "WOT": cvt(Wout[:, g * DL:(g + 1) * DL].T),
            "CORR": corr,
        })
    return in_maps, s_pad


def _run(in_maps, s_pad, trace=False):
    nc = bacc.Bacc("TRN2", target_bir_lowering=False, debug=False, num_devices=8)
    _build(nc, s_pad)
    res = run_bass_kernel_spmd(nc, in_maps, core_ids=list(range(8)), trace=trace)
    out = np.empty((B, N, DIM), np.float32)
    for b in range(B):
        out[b] = (res.results[2 * b]["OUT0"] + res.results[2 * b]["OUT1"]
                  + res.results[2 * b + 1]["OUT0"] + res.results[2 * b + 1]["OUT1"])
    return out, res


def kernel(input_feature, mask, Wq, Wk, Wv, Wout):
    in_maps, s_pad = _prep(input_feature, mask, Wq, Wk, Wv, Wout)
    out, _ = _run(in_maps, s_pad)
    return out
